# revision 4
# baseline (speedup 1.0000x reference)
"""Trainium2 Bass kernel for nn_BiLSTM_7928509628689.

Masked bidirectional LSTM over N=2048 ragged sequences (T=64, D=512, H=256),
returning concat of final fwd/bwd hidden states [N, 2H].

Strategy (8 NeuronCores, data-parallel over N, 256 seqs/core):
  * All state kept TRANSPOSED: hT/cT [H, Ns] folded into [128, 2*Ns] tiles,
    so the recurrent matmul contracts over H on partitions with no
    per-step transposes.
  * Per step s and direction, gates^T [4H, Ns] are built in PSUM by one
    accumulation group per gate bank [128, 2*Ns]:
       4 matmuls  W_ih^T chunks   @ x_s chunks    (input projection)
       2 matmuls  W_hh^T chunks   @ hT halves     (recurrence)
       1 matmul   [b; mask_coef]  @ [ones; maskinv_s]   (bias + masking)
    All matmuls in float32r (1 col/cycle on PE, ~1e-4 numerics).
  * Ragged masking is folded into the data/gates, so the time loop is
    mask-free:
      - fwd consumes right-aligned (host-shifted) sequences,
      - bwd consumes time-reversed sequences,
      - for pad steps (s < T - len) the i/f/o gate pre-activations get a
        -40 penalty via the bias matmul => new state is forced to ~0
        until the sequence starts; the final state at s=T-1 is exactly
        the masked-LSTM output for both directions.
  * ACT does the 4 gate nonlinearities + tanh(c); DVE does the 4
    elementwise products/adds; both hidden under PE.

kernel(**inputs) takes the FULL unsharded inputs and returns [2048, 512] f32.
"""
import numpy as np

import concourse.tile as tile
from concourse import bacc, mybir
from concourse.bass_utils import run_bass_kernel_spmd
import bass_rust

F32 = mybir.dt.float32
F32R = mybir.dt.float32r
AF = mybir.ActivationFunctionType
OP = mybir.AluOpType

N, T, D, H = 2048, 64, 512, 256
NCORES = 8
NS = N // NCORES           # 256 sequences per core
FH = 4 * H                 # 1024 gate rows
KD = D // 128              # 4 x-projection K chunks
KH = H // 128              # 2 h-projection K chunks
FORCE = -40.0              # gate penalty for pad steps
MB = 8                     # mask rhs block (steps per mask DMA)
DIRS = ("f", "b")

_NC_CACHE = {}


def _inst(r):
    return getattr(r, "ins", r)


def _build(t_steps):
    import contextlib

    nc = bacc.Bacc("TRN2", target_bir_lowering=False, debug=False)

    x_dram = {}
    wih_d, whh_d, bm_d, out_d = {}, {}, {}, {}
    for d in DIRS:
        # x stored [t, 128, KD, NS]: (p, k) <-> input dim  dd = 4*p + k
        x_dram[d] = nc.dram_tensor(
            f"x{d}", [t_steps, 128, KD, NS], F32R, kind="ExternalInput"
        ).ap()
        wih_d[d] = nc.dram_tensor(
            f"wih{d}", [128, KD, FH], F32R, kind="ExternalInput"
        ).ap()
        whh_d[d] = nc.dram_tensor(
            f"whh{d}", [128, KH, FH], F32R, kind="ExternalInput"
        ).ap()
        bm_d[d] = nc.dram_tensor(f"bm{d}", [2, FH], F32R, kind="ExternalInput").ap()
        out_d[d] = nc.dram_tensor(
            f"hT{d}", [128, KH * NS], F32R, kind="ExternalOutput"
        ).ap()
    mask_d = nc.dram_tensor(
        "maskrhs", [2, t_steps * NS], F32R, kind="ExternalInput"
    ).ap()
    zeros_d = nc.dram_tensor(
        "zeros", [128, KH * NS], F32R, kind="ExternalInput"
    ).ap()

    with tile.TileContext(nc) as tc:
        with contextlib.ExitStack() as ctx:
            wpool = ctx.enter_context(tc.tile_pool(name="w", bufs=1))
            xpool = ctx.enter_context(tc.tile_pool(name="x", bufs=3))
            mpool = ctx.enter_context(tc.tile_pool(name="mask", bufs=2))
            spool = ctx.enter_context(tc.tile_pool(name="state", bufs=2))
            apool = ctx.enter_context(tc.tile_pool(name="acts", bufs=2))
            pspool = ctx.enter_context(tc.tile_pool(name="ps", bufs=1, space="PSUM"))

            wih_t, whh_t, bm_t = {}, {}, {}
            for d in DIRS:
                wih_t[d] = wpool.tile([128, KD, FH], F32R, tag=f"wih_{d}", name=f"wih_{d}")
                nc.sync.dma_start(wih_t[d][:], wih_d[d][:])
                whh_t[d] = wpool.tile([128, KH, FH], F32R, tag=f"whh_{d}", name=f"whh_{d}")
                nc.sync.dma_start(whh_t[d][:], whh_d[d][:])
                bm_t[d] = wpool.tile([2, FH], F32R, tag=f"bm_{d}", name=f"bm_{d}")
                nc.sync.dma_start(bm_t[d][:], bm_d[d][:])

            h_t, c_t = {}, {}
            for d in DIRS:
                h_t[d] = spool.tile([128, KH * NS], F32R, tag=f"h_{d}", name=f"h_{d}")
                nc.sync.dma_start(h_t[d][:], zeros_d[:])
                c_t[d] = spool.tile([128, KH * NS], F32, tag=f"c_{d}", name=f"c_{d}")
                nc.sync.dma_start(c_t[d][:], zeros_d.bitcast(F32)[:])

            mtile = None
            for s in range(t_steps):
                if s % MB == 0:
                    mw = min(MB, t_steps - s) * NS
                    mtile = mpool.tile([2, MB * NS], F32R, tag="m", name="mtile")
                    nc.sync.dma_start(
                        mtile[:, :mw], mask_d[:, s * NS : s * NS + mw]
                    )
                mrhs = mtile[:, (s % MB) * NS : (s % MB + 1) * NS]

                for d in DIRS:
                    xt = xpool.tile([128, KD, NS], F32R, tag=f"x_{d}", name=f"x_{d}")
                    nc.sync.dma_start(xt[:], x_dram[d][s])

                    banks = []
                    for g in range(4):  # i, f, g, o
                        pst = pspool.tile(
                            [128, 2 * NS], F32, tag=f"ps_{d}_{g}", name=f"ps_{d}_{g}"
                        )
                        start_mm = None
                        for half in range(2):
                            m = g * 2 + half
                            o_ap = pst[:, half * NS : (half + 1) * NS]
                            msl = slice(m * 128, (m + 1) * 128)
                            r = nc.tensor.matmul(
                                o_ap,
                                wih_t[d][:, 0, msl],
                                xt[:, 0, :],
                                start=(half == 0),
                                stop=False,
                            )
                            if half == 0:
                                start_mm = _inst(r)
                            else:
                                # half-1 matmuls rely on the bank-wide
                                # has_written clear done by half-0's start
                                bass_rust.add_dep_helper(
                                    _inst(r),
                                    start_mm,
                                    sync=False,
                                    reason="psum bank group order",
                                )
                            for k in range(1, KD):
                                nc.tensor.matmul(
                                    o_ap, wih_t[d][:, k, msl], xt[:, k, :],
                                    start=False, stop=False,
                                )
                            for kk in range(KH):
                                nc.tensor.matmul(
                                    o_ap,
                                    whh_t[d][:, kk, msl],
                                    h_t[d][:, kk * NS : (kk + 1) * NS],
                                    start=False,
                                    stop=False,
                                )
                            nc.tensor.matmul(
                                o_ap, bm_t[d][:, msl], mrhs,
                                start=False, stop=(half == 1),
                            )
                        banks.append(pst)

                    si = apool.tile([128, 2 * NS], F32, tag=f"si_{d}", name=f"si_{d}")
                    nc.scalar.activation(si[:], banks[0][:], AF.Sigmoid)
                    sf = apool.tile([128, 2 * NS], F32, tag=f"sf_{d}", name=f"sf_{d}")
                    nc.scalar.activation(sf[:], banks[1][:], AF.Sigmoid)
                    tg = apool.tile([128, 2 * NS], F32, tag=f"tg_{d}", name=f"tg_{d}")
                    nc.scalar.activation(tg[:], banks[2][:], AF.Tanh)
                    so = apool.tile([128, 2 * NS], F32, tag=f"so_{d}", name=f"so_{d}")
                    nc.scalar.activation(so[:], banks[3][:], AF.Sigmoid)

                    t1 = apool.tile([128, 2 * NS], F32, tag=f"t1_{d}", name=f"t1_{d}")
                    nc.vector.tensor_tensor(t1[:], si[:], tg[:], OP.mult)
                    cn = spool.tile([128, 2 * NS], F32, tag=f"c_{d}", name=f"c_{d}")
                    nc.vector.tensor_tensor(cn[:], sf[:], c_t[d][:], OP.mult)
                    nc.vector.tensor_tensor(cn[:], cn[:], t1[:], OP.add)
                    tcn = apool.tile([128, 2 * NS], F32, tag=f"tc_{d}", name=f"tc_{d}")
                    nc.scalar.activation(tcn[:], cn[:], AF.Tanh)
                    hn = spool.tile([128, 2 * NS], F32R, tag=f"h_{d}", name=f"h_{d}")
                    nc.vector.tensor_tensor(hn[:], so[:], tcn[:], OP.mult)
                    h_t[d], c_t[d] = hn, cn

            for d in DIRS:
                nc.sync.dma_start(out_d[d][:], h_t[d][:])

    nc.compile()
    return nc


def _get_nc(t_steps):
    if t_steps not in _NC_CACHE:
        _NC_CACHE[t_steps] = _build(t_steps)
    return _NC_CACHE[t_steps]


def _prep_weights(W_ih, W_hh, b):
    """lhsT layouts for one direction."""
    wih = np.ascontiguousarray(
        W_ih.T.reshape(128, KD, FH), dtype=np.float32
    )  # (p, k) <-> dd = KD*p + k
    whh = np.ascontiguousarray(
        W_hh.T.reshape(KH, 128, FH).transpose(1, 0, 2), dtype=np.float32
    )  # (p, kk) <-> hrow = 128*kk + p
    coef = np.zeros(FH, np.float32)
    coef[: 2 * H] = FORCE       # i, f gates
    coef[3 * H :] = FORCE       # o gate
    bm = np.ascontiguousarray(np.stack([b.astype(np.float32), coef]))
    return wih, whh, bm


def _prep_core(seqs_c, lens_c, t_steps):
    """Per-core input arrays. seqs_c [NS, T, D], lens_c [NS]."""
    ns = seqs_c.shape[0]
    shift = T - lens_c  # pad steps per sequence
    src_t = np.arange(T)[None, :] - shift[:, None]            # [NS, T]
    valid = src_t >= 0
    gat = seqs_c[np.arange(ns)[:, None], np.clip(src_t, 0, T - 1)]
    xf = np.where(valid[..., None], gat, np.float32(0.0))     # right-aligned
    xb = seqs_c[:, ::-1, :]                                   # time-reversed

    def to_dev(x_ntd):
        # [NS, T, D] -> [t, 128, KD, NS] with dd = KD*p + k
        xt = np.ascontiguousarray(x_ntd[:, :t_steps].transpose(1, 2, 0))  # [t, D, NS]
        return np.ascontiguousarray(
            xt.reshape(t_steps, 128, KD, ns), dtype=np.float32
        )

    maskinv = (np.arange(T)[:, None] < shift[None, :]).astype(np.float32)  # [T, NS]
    maskrhs = np.ascontiguousarray(
        np.stack([np.ones((t_steps, ns), np.float32), maskinv[:t_steps]])
        .reshape(2, t_steps * ns)
    )
    return {"xf": to_dev(xf), "xb": to_dev(xb), "maskrhs": maskrhs}


def _unfold(hT):
    """[128, KH*NS] device tile -> [NS, H] h matrix."""
    h_rows = np.concatenate([hT[:, i * NS : (i + 1) * NS] for i in range(KH)], axis=0)
    return h_rows.T  # [NS, H]


def _run(inputs, t_steps=T, trace=False, **spmd_kwargs):
    all_embs = np.asarray(inputs["all_embs"], dtype=np.float32)
    lengths = np.asarray(inputs["lengths"]).astype(np.int64)
    starts = np.asarray(inputs["starts"]).astype(np.int64)

    if np.array_equal(starts, np.arange(N, dtype=np.int64) * T):
        seqs = all_embs.reshape(N, T, D)
    else:
        seqs = all_embs[starts[:, None] + np.arange(T)[None, :]]

    w = {}
    for d, (wi, wh, bb) in {
        "f": (inputs["W_ih_f"], inputs["W_hh_f"], inputs["b_f"]),
        "b": (inputs["W_ih_b"], inputs["W_hh_b"], inputs["b_b"]),
    }.items():
        w[d] = _prep_weights(
            np.asarray(wi, np.float32), np.asarray(wh, np.float32),
            np.asarray(bb, np.float32),
        )

    in_maps = []
    for ci in range(NCORES):
        sl = slice(ci * NS, (ci + 1) * NS)
        m = _prep_core(seqs[sl], lengths[sl], t_steps)
        in_maps.append(
            {
                "xf": m["xf"], "xb": m["xb"], "maskrhs": m["maskrhs"],
                "zeros": np.zeros((128, KH * NS), np.float32),
                "wihf": w["f"][0], "whhf": w["f"][1], "bmf": w["f"][2],
                "wihb": w["b"][0], "whhb": w["b"][1], "bmb": w["b"][2],
            }
        )

    nc = _get_nc(t_steps)
    res = run_bass_kernel_spmd(
        nc, in_maps, core_ids=list(range(NCORES)), trace=trace, **spmd_kwargs
    )

    out = np.empty((N, 2 * H), np.float32)
    for ci in range(NCORES):
        sl = slice(ci * NS, (ci + 1) * NS)
        out[sl, :H] = _unfold(res.results[ci]["hTf"])
        out[sl, H:] = _unfold(res.results[ci]["hTb"])
    return out, res


def kernel(**inputs) -> np.ndarray:
    out, _ = _run(inputs)
    return out


# revision 16
# speedup vs baseline: 2.4235x; 2.4235x over previous
"""Trainium2 Bass kernel for nn_BiLSTM_7928509628689.

Masked bidirectional LSTM over N=2048 ragged sequences (T=64, D=512, H=256),
returning concat of final fwd/bwd hidden states [N, 2H].

Strategy (8 NeuronCores, data-parallel over N, 256 seqs/core):
  * Sequences are globally sorted by length (desc) and dealt round-robin to
    cores, so all cores carry a near-identical length profile. All
    sequences are right-aligned in time (they END at the last step), so at
    step s only the V_s longest sequences are active. V_s is baked into
    the program: every matmul / ACT / DVE op at step s is trimmed to V_s
    columns. Mean length is ~T/2, so this halves the PE columns.
  * All state kept TRANSPOSED: hT/cT [H, Ns] folded into persistent
    [128, 2*Ns] tiles updated in place (never-yet-active columns stay 0).
  * Per step and direction, gates^T [4H, V_s] are built in one 4-bank PSUM
    tile (bank order g,i,f,o) by one accumulation group per bank:
       4 matmuls  W_ih^T chunks @ x_s chunks    (input projection)
       2 matmuls  W_hh^T chunks @ hT halves     (recurrence)
       1 matmul   [b; mask_coef] @ [ones; maskinv_s]  (bias + pad forcing)
    Operands bf16 (fp32 PSUM accumulation), K=128 for every matmul so
    LDWEIGHTS stays FWL-pipelined under the stream.
  * Pad forcing: columns included before their sequence's first step get
    -40 on the i/f/o pre-activations, so their state is forced to ~0 until
    the sequence starts; the final state at the last step is exactly the
    masked-LSTM output for both directions (bwd consumes the time-reversed
    sequence).
  * ACT: one tanh over the g bank, one sigmoid spanning the i,f,o banks,
    one tanh(c); DVE does the elementwise updates on exact active ranges.

kernel(**inputs) takes the FULL unsharded inputs and returns [2048, 512] f32.
"""
import numpy as np

import concourse.tile as tile
from concourse import bacc, mybir
from concourse.bass_utils import run_bass_kernel_spmd
import bass_rust

F32 = mybir.dt.float32
BF16 = mybir.dt.bfloat16
AF = mybir.ActivationFunctionType
OP = mybir.AluOpType

N, T, D, H = 2048, 64, 512, 256
NCORES = 8
NS = N // NCORES           # 256 sequences per core
FH = 4 * H                 # 1024 gate rows
KD = D // 128              # 4 x-projection K chunks
KH = H // 128              # 2 h-projection K chunks
FORCE = -40.0              # gate penalty for pad steps
MB = 8                     # mask rhs block (steps per mask DMA)
DIRS = ("f", "b")
# PSUM bank order within the [128, 4*512] gates tile; sigmoid spans i,f,o
BANK_MS = ((4, 5), (0, 1), (2, 3), (6, 7))   # g, i, f, o
BANK_OF = [b * 512 for b in range(4)]

_NC_CACHE = {}


def _inst(r):
    return getattr(r, "ins", r)


def _build(t_steps, V):
    import contextlib

    nc = bacc.Bacc("TRN2", target_bir_lowering=False, debug=False)

    x_dram = {}
    wih_d, whh_d, bm_d, out_d = {}, {}, {}, {}
    for d in DIRS:
        # x stored [t, 128, KD, NS]: (p, k) <-> input dim  dd = KD*p + k
        x_dram[d] = nc.dram_tensor(
            f"x{d}", [t_steps, 128, KD, NS], BF16, kind="ExternalInput"
        ).ap()
        wih_d[d] = nc.dram_tensor(
            f"wih{d}", [128, KD, FH], BF16, kind="ExternalInput"
        ).ap()
        whh_d[d] = nc.dram_tensor(
            f"whh{d}", [128, KH, FH], BF16, kind="ExternalInput"
        ).ap()
        bm_d[d] = nc.dram_tensor(f"bm{d}", [128, FH], BF16, kind="ExternalInput").ap()
        out_d[d] = nc.dram_tensor(
            f"hT{d}", [128, KH * NS], F32, kind="ExternalOutput"
        ).ap()
    mask_d = nc.dram_tensor(
        "maskrhs", [128, t_steps * NS], BF16, kind="ExternalInput"
    ).ap()

    with tile.TileContext(nc) as tc:
        with contextlib.ExitStack() as ctx:
            wpool = ctx.enter_context(tc.tile_pool(name="w", bufs=1))
            xpool = ctx.enter_context(tc.tile_pool(name="x", bufs=3))
            mpool = ctx.enter_context(tc.tile_pool(name="mask", bufs=2))
            spool = ctx.enter_context(tc.tile_pool(name="state", bufs=1))
            opool = ctx.enter_context(tc.tile_pool(name="outs", bufs=1))
            apool = ctx.enter_context(tc.tile_pool(name="acts", bufs=2))
            pspool = ctx.enter_context(tc.tile_pool(name="ps", bufs=1, space="PSUM"))

            wih_t, whh_t, bm_t = {}, {}, {}
            for d in DIRS:
                wih_t[d] = wpool.tile([128, KD, FH], BF16, tag=f"wih_{d}", name=f"wih_{d}")
                nc.gpsimd.dma_start(wih_t[d][:], wih_d[d][:])
                whh_t[d] = wpool.tile([128, KH, FH], BF16, tag=f"whh_{d}", name=f"whh_{d}")
                nc.gpsimd.dma_start(whh_t[d][:], whh_d[d][:])
                bm_t[d] = wpool.tile([128, FH], BF16, tag=f"bm_{d}", name=f"bm_{d}")
                nc.gpsimd.dma_start(bm_t[d][:], bm_d[d][:])

            # persistent state tiles, updated in place; inactive columns
            # stay zero from this init
            h_t, c_t = {}, {}
            for d in DIRS:
                h_t[d] = spool.tile([128, KH * NS], BF16, tag=f"h_{d}", name=f"h_{d}")
                nc.vector.memset(h_t[d][:], 0.0)
                c_t[d] = spool.tile([128, KH * NS], F32, tag=f"c_{d}", name=f"c_{d}")
                nc.vector.memset(c_t[d][:], 0.0)

            mtile = None
            for s in range(t_steps):
                v = int(V[s])
                if s % MB == 0:
                    mw = min(MB, t_steps - s) * NS
                    mtile = mpool.tile([128, MB * NS], BF16, tag="m", name="mtile")
                    nc.sync.dma_start(
                        mtile[:, :mw], mask_d[:, s * NS : s * NS + mw]
                    )
                mrhs = mtile[:, (s % MB) * NS : (s % MB) * NS + v]

                last = s == t_steps - 1
                for d in DIRS:
                    xt = xpool.tile([128, KD, NS], BF16, tag=f"x_{d}", name=f"x_{d}")
                    nc.sync.dma_start(xt[:, :, :v], x_dram[d][s][:, :, :v])

                    ps = pspool.tile(
                        [128, 4 * 512], F32, tag=f"ps_{d}", name=f"ps_{d}"
                    )
                    for b, ms in enumerate(BANK_MS):  # g, i, f, o
                        start_mm = None
                        for half in range(2):
                            m = ms[half]
                            o_ap = ps[:, BANK_OF[b] + half * NS : BANK_OF[b] + half * NS + v]
                            msl = slice(m * 128, (m + 1) * 128)
                            r = nc.tensor.matmul(
                                o_ap, wih_t[d][:, 0, msl], xt[:, 0, :v],
                                start=(half == 0), stop=False,
                            )
                            if half == 0:
                                start_mm = _inst(r)
                            else:
                                # half-1 matmuls rely on the bank-wide
                                # has_written clear done by half-0's start
                                bass_rust.add_dep_helper(
                                    _inst(r), start_mm, sync=False,
                                    reason="psum bank group order",
                                )
                            for k in range(1, KD):
                                nc.tensor.matmul(
                                    o_ap, wih_t[d][:, k, msl], xt[:, k, :v],
                                    start=False, stop=False,
                                )
                            for kk in range(KH):
                                nc.tensor.matmul(
                                    o_ap,
                                    whh_t[d][:, kk, msl],
                                    h_t[d][:, kk * NS : kk * NS + v],
                                    start=False, stop=False,
                                )
                            nc.tensor.matmul(
                                o_ap, bm_t[d][:, msl], mrhs,
                                start=False, stop=(half == 1),
                            )

                    # ACT: tanh on g bank; sigmoid split so i/f don't wait
                    # for the o-bank matmuls
                    gv = NS + v  # used width within a bank
                    tg = apool.tile([128, 512], F32, tag=f"tg_{d}", name=f"tg_{d}")
                    nc.scalar.activation(tg[:, :gv], ps[:, :gv], AF.Tanh)
                    si = apool.tile([128, 2 * 512], F32, tag=f"si_{d}", name=f"si_{d}")
                    nc.scalar.activation(
                        si[:, : 512 + gv], ps[:, 512 : 2 * 512 + gv], AF.Sigmoid
                    )
                    so = apool.tile([128, 512], F32, tag=f"so_{d}", name=f"so_{d}")
                    nc.scalar.activation(so[:, :gv], ps[:, 3 * 512 : 3 * 512 + gv], AF.Sigmoid)

                    t1 = apool.tile([128, 512], F32, tag=f"t1_{d}", name=f"t1_{d}")
                    cc = c_t[d]
                    for r0 in (0, NS):
                        rr = slice(r0, r0 + v)
                        nc.vector.tensor_tensor(t1[:, rr], si[:, rr], tg[:, rr], OP.mult)
                        nc.vector.tensor_tensor(
                            cc[:, rr], si[:, 512 + r0 : 512 + r0 + v], cc[:, rr], OP.mult
                        )
                        nc.vector.tensor_tensor(cc[:, rr], cc[:, rr], t1[:, rr], OP.add)
                    tcn = apool.tile([128, 512], F32, tag=f"tc_{d}", name=f"tc_{d}")
                    nc.scalar.activation(tcn[:, :gv], cc[:, :gv], AF.Tanh)
                    if last:
                        hf = opool.tile([128, 512], F32, tag=f"hout_{d}", name=f"hout_{d}")
                        nc.vector.tensor_tensor(hf[:], so[:], tcn[:], OP.mult)
                        nc.sync.dma_start(out_d[d][:], hf[:])
                    else:
                        hh = h_t[d]
                        for r0 in (0, NS):
                            rr = slice(r0, r0 + v)
                            nc.vector.tensor_tensor(
                                hh[:, rr], so[:, rr], tcn[:, rr], OP.mult,
                            )

    nc.compile()
    return nc


def _get_nc(t_steps, V):
    key = (t_steps, tuple(V))
    if key not in _NC_CACHE:
        _NC_CACHE[key] = _build(t_steps, V)
    return _NC_CACHE[key]


def _prep_weights(W_ih, W_hh, b):
    """lhsT layouts for one direction."""
    import ml_dtypes

    wdt = ml_dtypes.bfloat16
    wih = np.ascontiguousarray(
        W_ih.T.reshape(128, KD, FH).astype(wdt)
    )  # (p, k) <-> dd = KD*p + k
    whh = np.ascontiguousarray(
        W_hh.T.reshape(KH, 128, FH).transpose(1, 0, 2).astype(wdt)
    )  # (p, kk) <-> hrow = 128*kk + p
    coef = np.zeros(FH, np.float32)
    coef[: 2 * H] = FORCE       # i, f gates
    coef[3 * H :] = FORCE       # o gate
    bm = np.zeros((128, FH), np.float32)
    bm[0] = b.astype(np.float32)
    bm[1] = coef
    bm = np.ascontiguousarray(bm.astype(wdt))
    return wih, whh, bm


def _prep_core(seqs_c, lens_c, t_steps):
    """Per-core device arrays. seqs_c [NS, T, D], lens_c [NS] (sorted desc)."""
    import ml_dtypes

    bf16 = ml_dtypes.bfloat16
    ns = seqs_c.shape[0]
    shift = t_steps - lens_c  # pad steps per sequence
    src_t = np.arange(t_steps)[None, :] - shift[:, None]      # [NS, t]
    valid = src_t >= 0
    gat = seqs_c[np.arange(ns)[:, None], np.clip(src_t, 0, T - 1)]
    xf = np.where(valid[..., None], gat, np.float32(0.0))     # right-aligned
    xb = seqs_c[:, t_steps - 1 :: -1, :]                      # time-reversed

    def to_dev(x_ntd):
        # [NS, t, D] -> [t, 128, KD, NS] with dd = KD*p + k
        xt = x_ntd.transpose(1, 2, 0).astype(bf16)            # [t, D, NS]
        return np.ascontiguousarray(xt.reshape(t_steps, 128, KD, ns))

    maskinv = (np.arange(t_steps)[:, None] < shift[None, :]).astype(np.float32)
    maskrhs = np.zeros((128, t_steps * ns), np.float32)
    maskrhs[0] = 1.0
    maskrhs[1] = maskinv.reshape(t_steps * ns)
    maskrhs = np.ascontiguousarray(maskrhs.astype(bf16))
    return {"xf": to_dev(xf), "xb": to_dev(xb), "maskrhs": maskrhs}


def _unfold(hT):
    """[128, KH*NS] device tile -> [NS, H] h matrix."""
    h_rows = np.concatenate([hT[:, i * NS : (i + 1) * NS] for i in range(KH)], axis=0)
    return h_rows.T  # [NS, H]


def _run(inputs, trace=False, t_cap=None, **spmd_kwargs):
    import ml_dtypes

    all_embs = np.asarray(inputs["all_embs"], dtype=np.float32)
    lengths = np.asarray(inputs["lengths"]).astype(np.int64)
    starts = np.asarray(inputs["starts"]).astype(np.int64)

    if np.array_equal(starts, np.arange(N, dtype=np.int64) * T):
        seqs = all_embs.reshape(N, T, D)
    else:
        seqs = all_embs[starts[:, None] + np.arange(T)[None, :]]

    # global sort by length desc, deal round-robin to cores
    order = np.argsort(-lengths, kind="stable")
    t_steps = int(lengths.max())
    if t_cap is not None:
        t_steps = min(t_steps, t_cap)
    core_idx = [order[c::NCORES] for c in range(NCORES)]  # [NCORES][NS]

    # baked active widths: V_s = max over cores of #{len >= t_steps - s}
    Ls = np.stack([np.minimum(lengths[ci], t_steps) for ci in core_idx])  # [NC, NS]
    thr = t_steps - np.arange(t_steps)  # [t]
    V = (Ls[:, None, :] >= thr[None, :, None]).sum(-1).max(0)  # [t]
    V = np.maximum(V, 1)

    w = {}
    for d, (wi, wh, bb) in {
        "f": (inputs["W_ih_f"], inputs["W_hh_f"], inputs["b_f"]),
        "b": (inputs["W_ih_b"], inputs["W_hh_b"], inputs["b_b"]),
    }.items():
        w[d] = _prep_weights(
            np.asarray(wi, np.float32), np.asarray(wh, np.float32),
            np.asarray(bb, np.float32),
        )

    in_maps = []
    for ci in range(NCORES):
        idx = core_idx[ci]
        m = _prep_core(seqs[idx], np.minimum(lengths[idx], t_steps), t_steps)
        in_maps.append(
            {
                "xf": m["xf"], "xb": m["xb"], "maskrhs": m["maskrhs"],
                "wihf": w["f"][0], "whhf": w["f"][1], "bmf": w["f"][2],
                "wihb": w["b"][0], "whhb": w["b"][1], "bmb": w["b"][2],
            }
        )

    nc = _get_nc(t_steps, V)
    res = None
    for attempt in range(3):
        try:
            res = run_bass_kernel_spmd(
                nc, in_maps, core_ids=list(range(NCORES)), trace=trace,
                **spmd_kwargs
            )
            break
        except Exception:
            # rare transient NRT_EXEC_UNIT_UNRECOVERABLE right after a
            # fresh NEFF load; a plain re-execute has always recovered
            if attempt == 2:
                raise
            import time as _time

            _time.sleep(2.0)

    out = np.empty((N, 2 * H), np.float32)
    for ci in range(NCORES):
        out[core_idx[ci], :H] = _unfold(res.results[ci]["hTf"])
        out[core_idx[ci], H:] = _unfold(res.results[ci]["hTb"])
    return out, res


def kernel(**inputs) -> np.ndarray:
    out, _ = _run(inputs)
    return out


# revision 17
# speedup vs baseline: 2.6875x; 1.1089x over previous
"""Trainium2 Bass kernel for nn_BiLSTM_7928509628689.

Masked bidirectional LSTM over N=2048 ragged sequences (T=64, D=512, H=256),
returning concat of final fwd/bwd hidden states [N, 2H].

Strategy (8 NeuronCores, data-parallel over N, 256 seqs/core):
  * Sequences are globally sorted by length (desc) and dealt round-robin to
    cores, so all cores carry a near-identical length profile. All
    sequences are right-aligned in time (they END at the last step), so at
    step s only the V_s longest sequences are active. V_s is baked into
    the program: every matmul / ACT / DVE op at step s is trimmed to V_s
    columns. Mean length is ~T/2, so this halves the PE columns.
  * All state kept TRANSPOSED: hT/cT [H, Ns] folded into persistent
    [128, 2*Ns] tiles updated in place (never-yet-active columns stay 0).
  * Per step and direction, gates^T [4H, V_s] are built in one 4-bank PSUM
    tile (bank order g,i,f,o) by one accumulation group per bank:
       4 matmuls  W_ih^T chunks @ x_s chunks    (input projection)
       2 matmuls  W_hh^T chunks @ hT halves     (recurrence)
       1 matmul   [b; mask_coef] @ [ones; maskinv_s]  (bias + pad forcing)
    Operands bf16 (fp32 PSUM accumulation), K=128 for every matmul so
    LDWEIGHTS stays FWL-pipelined under the stream.
  * Pad forcing: columns included before their sequence's first step get
    -40 on the i/f/o pre-activations, so their state is forced to ~0 until
    the sequence starts; the final state at the last step is exactly the
    masked-LSTM output for both directions (bwd consumes the time-reversed
    sequence).
  * ACT: one tanh over the g bank, one sigmoid spanning the i,f,o banks,
    one tanh(c); DVE does the elementwise updates on exact active ranges.

kernel(**inputs) takes the FULL unsharded inputs and returns [2048, 512] f32.
"""
import numpy as np

import concourse.tile as tile
from concourse import bacc, mybir
from concourse.bass_utils import run_bass_kernel_spmd
import bass_rust

F32 = mybir.dt.float32
BF16 = mybir.dt.bfloat16
AF = mybir.ActivationFunctionType
OP = mybir.AluOpType

N, T, D, H = 2048, 64, 512, 256
NCORES = 8
NS = N // NCORES           # 256 sequences per core
FH = 4 * H                 # 1024 gate rows
KD = D // 128              # 4 x-projection K chunks
KH = H // 128              # 2 h-projection K chunks
FORCE = -40.0              # gate penalty for pad steps
MB = 8                     # mask rhs block (steps per mask DMA)
DIRS = ("f", "b")
# PSUM bank order within the [128, 4*512] gates tile; sigmoid spans i,f,o
BANK_MS = ((4, 5), (0, 1), (2, 3), (6, 7))   # g, i, f, o
BANK_OF = [b * 512 for b in range(4)]

_NC_CACHE = {}


def _inst(r):
    return getattr(r, "ins", r)


def _build(t_steps, V):
    import contextlib

    nc = bacc.Bacc("TRN2", target_bir_lowering=False, debug=False)

    x_dram = {}
    wih_d, whh_d, bm_d, out_d = {}, {}, {}, {}
    for d in DIRS:
        # x stored [t, 128, KD, NS]: (p, k) <-> input dim  dd = KD*p + k
        x_dram[d] = nc.dram_tensor(
            f"x{d}", [t_steps, 128, KD, NS], BF16, kind="ExternalInput"
        ).ap()
        wih_d[d] = nc.dram_tensor(
            f"wih{d}", [128, KD, FH], BF16, kind="ExternalInput"
        ).ap()
        whh_d[d] = nc.dram_tensor(
            f"whh{d}", [128, KH, FH], BF16, kind="ExternalInput"
        ).ap()
        bm_d[d] = nc.dram_tensor(f"bm{d}", [128, FH], BF16, kind="ExternalInput").ap()
        out_d[d] = nc.dram_tensor(
            f"hT{d}", [128, KH * NS], F32, kind="ExternalOutput"
        ).ap()
    mask_d = nc.dram_tensor(
        "maskrhs", [128, t_steps * NS], BF16, kind="ExternalInput"
    ).ap()

    with tile.TileContext(nc) as tc:
        with contextlib.ExitStack() as ctx:
            wpool = ctx.enter_context(tc.tile_pool(name="w", bufs=1))
            xpool = ctx.enter_context(tc.tile_pool(name="x", bufs=3))
            mpool = ctx.enter_context(tc.tile_pool(name="mask", bufs=2))
            spool = ctx.enter_context(tc.tile_pool(name="state", bufs=1))
            opool = ctx.enter_context(tc.tile_pool(name="outs", bufs=1))
            apool = ctx.enter_context(tc.tile_pool(name="acts", bufs=2))
            pspool = ctx.enter_context(tc.tile_pool(name="ps", bufs=1, space="PSUM"))

            wih_t, whh_t, bm_t = {}, {}, {}
            for d in DIRS:
                wih_t[d] = wpool.tile([128, KD, FH], BF16, tag=f"wih_{d}", name=f"wih_{d}")
                nc.gpsimd.dma_start(wih_t[d][:], wih_d[d][:])
                whh_t[d] = wpool.tile([128, KH, FH], BF16, tag=f"whh_{d}", name=f"whh_{d}")
                nc.gpsimd.dma_start(whh_t[d][:], whh_d[d][:])
                bm_t[d] = wpool.tile([128, FH], BF16, tag=f"bm_{d}", name=f"bm_{d}")
                nc.gpsimd.dma_start(bm_t[d][:], bm_d[d][:])

            # persistent state tiles, updated in place; inactive columns
            # stay zero from this init
            h_t, c_t = {}, {}
            for d in DIRS:
                h_t[d] = spool.tile([128, KH * NS], BF16, tag=f"h_{d}", name=f"h_{d}")
                nc.vector.memset(h_t[d][:], 0.0)
                c_t[d] = spool.tile([128, KH * NS], F32, tag=f"c_{d}", name=f"c_{d}")
                nc.vector.memset(c_t[d][:], 0.0)

            # PE warm-up burst: dense dummy matmuls during the initial
            # weight/x DMA window so HAM reaches full clock before step 0
            wrm = wpool.tile([128, 512], BF16, tag="warm", name="warm")
            nc.vector.memset(wrm[:], 0.0)
            wps = pspool.tile([128, 4 * 512], F32, tag="ps_f", name="warm_ps")
            NWARM = 40
            for i in range(NWARM):
                nc.tensor.matmul(
                    wps[:, 0:512], wrm[:, 0:128], wrm[:],
                    start=(i == 0), stop=(i == NWARM - 1),
                )

            mtile = None
            for s in range(t_steps):
                v = int(V[s])
                xts = {}
                for d in DIRS:
                    xts[d] = xpool.tile(
                        [128, KD, NS], BF16, tag=f"x_{d}", name=f"x_{d}"
                    )
                    nc.sync.dma_start(xts[d][:, :, :v], x_dram[d][s][:, :, :v])
                if s % MB == 0:
                    mw = min(MB, t_steps - s) * NS
                    mtile = mpool.tile([128, MB * NS], BF16, tag="m", name="mtile")
                    nc.sync.dma_start(
                        mtile[:, :mw], mask_d[:, s * NS : s * NS + mw]
                    )
                mrhs = mtile[:, (s % MB) * NS : (s % MB) * NS + v]

                last = s == t_steps - 1
                for d in DIRS:
                    xt = xts[d]

                    ps = pspool.tile(
                        [128, 4 * 512], F32, tag=f"ps_{d}", name=f"ps_{d}"
                    )
                    for b, ms in enumerate(BANK_MS):  # g, i, f, o
                        start_mm = None
                        for half in range(2):
                            m = ms[half]
                            o_ap = ps[:, BANK_OF[b] + half * NS : BANK_OF[b] + half * NS + v]
                            msl = slice(m * 128, (m + 1) * 128)
                            r = nc.tensor.matmul(
                                o_ap, wih_t[d][:, 0, msl], xt[:, 0, :v],
                                start=(half == 0), stop=False,
                            )
                            if half == 0:
                                start_mm = _inst(r)
                            else:
                                # half-1 matmuls rely on the bank-wide
                                # has_written clear done by half-0's start
                                bass_rust.add_dep_helper(
                                    _inst(r), start_mm, sync=False,
                                    reason="psum bank group order",
                                )
                            for k in range(1, KD):
                                nc.tensor.matmul(
                                    o_ap, wih_t[d][:, k, msl], xt[:, k, :v],
                                    start=False, stop=False,
                                )
                            for kk in range(KH):
                                nc.tensor.matmul(
                                    o_ap,
                                    whh_t[d][:, kk, msl],
                                    h_t[d][:, kk * NS : kk * NS + v],
                                    start=False, stop=False,
                                )
                            nc.tensor.matmul(
                                o_ap, bm_t[d][:, msl], mrhs,
                                start=False, stop=(half == 1),
                            )

                    # ACT/DVE on strided 2-range views that skip the dead
                    # gap between the two half-ranges of each bank
                    def v2(ap, q):
                        return ap.rearrange("p (q n) -> p q n", q=q)[:, :, :v]

                    tg = apool.tile([128, 512], F32, tag=f"tg_{d}", name=f"tg_{d}")
                    nc.scalar.activation(v2(tg[:], 2), v2(ps[:, 0:512], 2), AF.Tanh)
                    si = apool.tile([128, 2 * 512], F32, tag=f"si_{d}", name=f"si_{d}")
                    nc.scalar.activation(
                        v2(si[:], 4), v2(ps[:, 512 : 3 * 512], 4), AF.Sigmoid
                    )
                    so = apool.tile([128, 512], F32, tag=f"so_{d}", name=f"so_{d}")
                    nc.scalar.activation(
                        v2(so[:], 2), v2(ps[:, 3 * 512 :], 2), AF.Sigmoid
                    )

                    t1 = apool.tile([128, 512], F32, tag=f"t1_{d}", name=f"t1_{d}")
                    cc = c_t[d]
                    nc.vector.tensor_tensor(
                        v2(t1[:], 2), v2(si[:, 0:512], 2), v2(tg[:], 2), OP.mult
                    )
                    nc.vector.tensor_tensor(
                        v2(cc[:], 2), v2(si[:, 512:1024], 2), v2(cc[:], 2), OP.mult
                    )
                    nc.vector.tensor_tensor(
                        v2(cc[:], 2), v2(cc[:], 2), v2(t1[:], 2), OP.add
                    )
                    tcn = apool.tile([128, 512], F32, tag=f"tc_{d}", name=f"tc_{d}")
                    nc.scalar.activation(v2(tcn[:], 2), v2(cc[:], 2), AF.Tanh)
                    if last:
                        hf = opool.tile([128, 512], F32, tag=f"hout_{d}", name=f"hout_{d}")
                        nc.vector.tensor_tensor(hf[:], so[:], tcn[:], OP.mult)
                        nc.sync.dma_start(out_d[d][:], hf[:])
                    else:
                        nc.vector.tensor_tensor(
                            v2(h_t[d][:], 2), v2(so[:], 2), v2(tcn[:], 2), OP.mult
                        )

    nc.compile()
    return nc


def _get_nc(t_steps, V):
    key = (t_steps, tuple(V))
    if key not in _NC_CACHE:
        _NC_CACHE[key] = _build(t_steps, V)
    return _NC_CACHE[key]


def _prep_weights(W_ih, W_hh, b):
    """lhsT layouts for one direction."""
    import ml_dtypes

    wdt = ml_dtypes.bfloat16
    wih = np.ascontiguousarray(
        W_ih.T.reshape(128, KD, FH).astype(wdt)
    )  # (p, k) <-> dd = KD*p + k
    whh = np.ascontiguousarray(
        W_hh.T.reshape(KH, 128, FH).transpose(1, 0, 2).astype(wdt)
    )  # (p, kk) <-> hrow = 128*kk + p
    coef = np.zeros(FH, np.float32)
    coef[: 2 * H] = FORCE       # i, f gates
    coef[3 * H :] = FORCE       # o gate
    bm = np.zeros((128, FH), np.float32)
    bm[0] = b.astype(np.float32)
    bm[1] = coef
    bm = np.ascontiguousarray(bm.astype(wdt))
    return wih, whh, bm


def _prep_core(seqs_c, lens_c, t_steps):
    """Per-core device arrays. seqs_c [NS, T, D], lens_c [NS] (sorted desc)."""
    import ml_dtypes

    bf16 = ml_dtypes.bfloat16
    ns = seqs_c.shape[0]
    shift = t_steps - lens_c  # pad steps per sequence
    src_t = np.arange(t_steps)[None, :] - shift[:, None]      # [NS, t]
    valid = src_t >= 0
    gat = seqs_c[np.arange(ns)[:, None], np.clip(src_t, 0, T - 1)]
    xf = np.where(valid[..., None], gat, np.float32(0.0))     # right-aligned
    xb = seqs_c[:, t_steps - 1 :: -1, :]                      # time-reversed

    def to_dev(x_ntd):
        # [NS, t, D] -> [t, 128, KD, NS] with dd = KD*p + k
        xt = x_ntd.transpose(1, 2, 0).astype(bf16)            # [t, D, NS]
        return np.ascontiguousarray(xt.reshape(t_steps, 128, KD, ns))

    maskinv = (np.arange(t_steps)[:, None] < shift[None, :]).astype(np.float32)
    maskrhs = np.zeros((128, t_steps * ns), np.float32)
    maskrhs[0] = 1.0
    maskrhs[1] = maskinv.reshape(t_steps * ns)
    maskrhs = np.ascontiguousarray(maskrhs.astype(bf16))
    return {"xf": to_dev(xf), "xb": to_dev(xb), "maskrhs": maskrhs}


def _unfold(hT):
    """[128, KH*NS] device tile -> [NS, H] h matrix."""
    h_rows = np.concatenate([hT[:, i * NS : (i + 1) * NS] for i in range(KH)], axis=0)
    return h_rows.T  # [NS, H]


def _run(inputs, trace=False, t_cap=None, **spmd_kwargs):
    import ml_dtypes

    all_embs = np.asarray(inputs["all_embs"], dtype=np.float32)
    lengths = np.asarray(inputs["lengths"]).astype(np.int64)
    starts = np.asarray(inputs["starts"]).astype(np.int64)

    if np.array_equal(starts, np.arange(N, dtype=np.int64) * T):
        seqs = all_embs.reshape(N, T, D)
    else:
        seqs = all_embs[starts[:, None] + np.arange(T)[None, :]]

    # global sort by length desc, deal round-robin to cores
    order = np.argsort(-lengths, kind="stable")
    t_steps = int(lengths.max())
    if t_cap is not None:
        t_steps = min(t_steps, t_cap)
    core_idx = [order[c::NCORES] for c in range(NCORES)]  # [NCORES][NS]

    # baked active widths: V_s = max over cores of #{len >= t_steps - s}
    Ls = np.stack([np.minimum(lengths[ci], t_steps) for ci in core_idx])  # [NC, NS]
    thr = t_steps - np.arange(t_steps)  # [t]
    V = (Ls[:, None, :] >= thr[None, :, None]).sum(-1).max(0)  # [t]
    V = np.maximum(V, 1)

    w = {}
    for d, (wi, wh, bb) in {
        "f": (inputs["W_ih_f"], inputs["W_hh_f"], inputs["b_f"]),
        "b": (inputs["W_ih_b"], inputs["W_hh_b"], inputs["b_b"]),
    }.items():
        w[d] = _prep_weights(
            np.asarray(wi, np.float32), np.asarray(wh, np.float32),
            np.asarray(bb, np.float32),
        )

    in_maps = []
    for ci in range(NCORES):
        idx = core_idx[ci]
        m = _prep_core(seqs[idx], np.minimum(lengths[idx], t_steps), t_steps)
        in_maps.append(
            {
                "xf": m["xf"], "xb": m["xb"], "maskrhs": m["maskrhs"],
                "wihf": w["f"][0], "whhf": w["f"][1], "bmf": w["f"][2],
                "wihb": w["b"][0], "whhb": w["b"][1], "bmb": w["b"][2],
            }
        )

    nc = _get_nc(t_steps, V)
    res = None
    for attempt in range(3):
        try:
            res = run_bass_kernel_spmd(
                nc, in_maps, core_ids=list(range(NCORES)), trace=trace,
                **spmd_kwargs
            )
            break
        except Exception:
            # rare transient NRT_EXEC_UNIT_UNRECOVERABLE right after a
            # fresh NEFF load; a plain re-execute has always recovered
            if attempt == 2:
                raise
            import time as _time

            _time.sleep(2.0)

    out = np.empty((N, 2 * H), np.float32)
    for ci in range(NCORES):
        out[core_idx[ci], :H] = _unfold(res.results[ci]["hTf"])
        out[core_idx[ci], H:] = _unfold(res.results[ci]["hTb"])
    return out, res


def kernel(**inputs) -> np.ndarray:
    out, _ = _run(inputs)
    return out


# revision 18
# speedup vs baseline: 2.8359x; 1.0552x over previous
"""Trainium2 Bass kernel for nn_BiLSTM_7928509628689.

Masked bidirectional LSTM over N=2048 ragged sequences (T=64, D=512, H=256),
returning concat of final fwd/bwd hidden states [N, 2H].

Strategy (8 NeuronCores, data-parallel over N, 256 seqs/core):
  * Sequences are globally sorted by length (desc) and dealt round-robin to
    cores, so all cores carry a near-identical length profile. All
    sequences are right-aligned in time (they END at the last step), so at
    step s only the V_s longest sequences are active. V_s is baked into
    the program: every matmul / ACT / DVE op at step s is trimmed to V_s
    columns. Mean length is ~T/2, so this halves the PE columns.
  * All state kept TRANSPOSED: hT/cT [H, Ns] folded into persistent
    [128, 2*Ns] tiles updated in place (never-yet-active columns stay 0).
  * Per step and direction, gates^T [4H, V_s] are built in one 4-bank PSUM
    tile (bank order g,i,f,o) by one accumulation group per bank:
       4 matmuls  W_ih^T chunks @ x_s chunks    (input projection)
       2 matmuls  W_hh^T chunks @ hT halves     (recurrence)
       1 matmul   [b; mask_coef] @ [ones; maskinv_s]  (bias + pad forcing)
    Operands bf16 (fp32 PSUM accumulation), K=128 for every matmul so
    LDWEIGHTS stays FWL-pipelined under the stream.
  * Pad forcing: columns included before their sequence's first step get
    -40 on the i/f/o pre-activations, so their state is forced to ~0 until
    the sequence starts; the final state at the last step is exactly the
    masked-LSTM output for both directions (bwd consumes the time-reversed
    sequence).
  * ACT: one tanh over the g bank, one sigmoid spanning the i,f,o banks,
    one tanh(c); DVE does the elementwise updates on exact active ranges.

kernel(**inputs) takes the FULL unsharded inputs and returns [2048, 512] f32.
"""
import numpy as np

import concourse.tile as tile
from concourse import bacc, mybir
from concourse.bass_utils import run_bass_kernel_spmd
import bass_rust

F32 = mybir.dt.float32
BF16 = mybir.dt.bfloat16
AF = mybir.ActivationFunctionType
OP = mybir.AluOpType

N, T, D, H = 2048, 64, 512, 256
NCORES = 8
NS = N // NCORES           # 256 sequences per core
FH = 4 * H                 # 1024 gate rows
KD = D // 128              # 4 x-projection K chunks
KH = H // 128              # 2 h-projection K chunks
FORCE = -40.0              # gate penalty for pad steps
MB = 8                     # mask rhs block (steps per mask DMA)
DIRS = ("f", "b")
# PSUM bank order within the [128, 4*512] gates tile; sigmoid spans i,f,o
BANK_MS = ((4, 5), (0, 1), (2, 3), (6, 7))   # g, i, f, o
BANK_OF = [b * 512 for b in range(4)]

_NC_CACHE = {}


def _inst(r):
    return getattr(r, "ins", r)


def _build(t_steps, V):
    import contextlib

    nc = bacc.Bacc("TRN2", target_bir_lowering=False, debug=False)

    x_dram = {}
    wih_d, whh_d, bm_d, out_d = {}, {}, {}, {}
    for d in DIRS:
        # x stored [t, 128, KD, NS]: (p, k) <-> input dim  dd = KD*p + k
        x_dram[d] = nc.dram_tensor(
            f"x{d}", [t_steps, 128, KD, NS], BF16, kind="ExternalInput"
        ).ap()
        wih_d[d] = nc.dram_tensor(
            f"wih{d}", [128, KD, FH], BF16, kind="ExternalInput"
        ).ap()
        whh_d[d] = nc.dram_tensor(
            f"whh{d}", [128, KH, FH], BF16, kind="ExternalInput"
        ).ap()
        bm_d[d] = nc.dram_tensor(f"bm{d}", [128, FH], BF16, kind="ExternalInput").ap()
        out_d[d] = nc.dram_tensor(
            f"hT{d}", [128, KH * NS], F32, kind="ExternalOutput"
        ).ap()
    mask_d = nc.dram_tensor(
        "maskrhs", [128, t_steps * NS], BF16, kind="ExternalInput"
    ).ap()

    with tile.TileContext(nc) as tc:
        with contextlib.ExitStack() as ctx:
            wpool = ctx.enter_context(tc.tile_pool(name="w", bufs=1))
            xpool = ctx.enter_context(tc.tile_pool(name="x", bufs=3))
            mpool = ctx.enter_context(tc.tile_pool(name="mask", bufs=2))
            spool = ctx.enter_context(tc.tile_pool(name="state", bufs=1))
            opool = ctx.enter_context(tc.tile_pool(name="outs", bufs=1))
            apool = ctx.enter_context(tc.tile_pool(name="acts", bufs=2))
            pspool = ctx.enter_context(tc.tile_pool(name="ps", bufs=1, space="PSUM"))

            wih_t, whh_t, bm_t = {}, {}, {}
            for d in DIRS:
                wih_t[d] = wpool.tile([128, KD, FH], BF16, tag=f"wih_{d}", name=f"wih_{d}")
                nc.gpsimd.dma_start(wih_t[d][:], wih_d[d][:])
                whh_t[d] = wpool.tile([128, KH, FH], BF16, tag=f"whh_{d}", name=f"whh_{d}")
                nc.gpsimd.dma_start(whh_t[d][:], whh_d[d][:])
                bm_t[d] = wpool.tile([128, FH], BF16, tag=f"bm_{d}", name=f"bm_{d}")
                nc.gpsimd.dma_start(bm_t[d][:], bm_d[d][:])

            # persistent state tiles, updated in place; inactive columns
            # stay zero from this init
            h_t, c_t = {}, {}
            for d in DIRS:
                h_t[d] = spool.tile([128, KH * NS], BF16, tag=f"h_{d}", name=f"h_{d}")
                nc.vector.memset(h_t[d][:], 0.0)
                c_t[d] = spool.tile([128, KH * NS], F32, tag=f"c_{d}", name=f"c_{d}")
                nc.vector.memset(c_t[d][:], 0.0)

            # persistent per-direction gates tiles (4 PSUM banks each);
            # bank-granular dep tracking lets a step's matmuls overlap the
            # previous step's ACT reads when they touch different banks
            ps_t = {}
            for d in DIRS:
                ps_t[d] = pspool.tile(
                    [128, 4 * 512], F32, tag=f"ps_{d}", name=f"ps_{d}"
                )

            # PE warm-up burst: dense dummy matmuls during the initial
            # weight/x DMA window so HAM reaches full clock before step 0
            wrm = wpool.tile([128, 512], BF16, tag="warm", name="warm")
            nc.vector.memset(wrm[:], 0.0)
            NWARM = 40
            for i in range(NWARM):
                nc.tensor.matmul(
                    ps_t["f"][:, 0:512], wrm[:, 0:128], wrm[:],
                    start=(i == 0), stop=(i == NWARM - 1),
                )

            mtile = None
            for s in range(t_steps):
                v = int(V[s])
                xts = {}
                for d in DIRS:
                    xts[d] = xpool.tile(
                        [128, KD, NS], BF16, tag=f"x_{d}", name=f"x_{d}"
                    )
                    nc.sync.dma_start(xts[d][:, :, :v], x_dram[d][s][:, :, :v])
                if s % MB == 0:
                    mw = min(MB, t_steps - s) * NS
                    mtile = mpool.tile([128, MB * NS], BF16, tag="m", name="mtile")
                    nc.sync.dma_start(
                        mtile[:, :mw], mask_d[:, s * NS : s * NS + mw]
                    )
                mrhs = mtile[:, (s % MB) * NS : (s % MB) * NS + v]

                last = s == t_steps - 1
                packed = v <= 128
                for d in DIRS:
                    xt = xts[d]
                    ps = ps_t[d]

                    if packed:
                        # all 8 half-ranges in 2 banks; alternate bank set
                        # by step parity so the next step's matmuls overlap
                        # this step's ACT reads (different banks)
                        base = 1024 * (s % 2)
                        banks = [
                            [(base + j * 128, m) for j, m in enumerate((4, 5, 0, 1))],
                            [(base + 512 + j * 128, m) for j, m in enumerate((2, 3, 6, 7))],
                        ]
                        tg_sl = ps[:, base : base + 256]
                        si_sl = ps[:, base + 256 : base + 768]
                        so_sl = ps[:, base + 768 : base + 1024]
                    else:
                        banks = [
                            [(b * 512 + half * NS, BANK_MS[b][half]) for half in range(2)]
                            for b in range(4)
                        ]
                        tg_sl = ps[:, 0:512]
                        si_sl = ps[:, 512 : 3 * 512]
                        so_sl = ps[:, 3 * 512 :]

                    # pass 1: x-projection + bias/mask (independent of h),
                    # one accumulation group per physical bank
                    starts = []
                    for regs in banks:
                        start_mm = None
                        for idx, (off, m) in enumerate(regs):
                            o_ap = ps[:, off : off + v]
                            msl = slice(m * 128, (m + 1) * 128)
                            r = nc.tensor.matmul(
                                o_ap, wih_t[d][:, 0, msl], xt[:, 0, :v],
                                start=(idx == 0), stop=False,
                            )
                            if idx == 0:
                                start_mm = _inst(r)
                            else:
                                # later regions rely on the bank-wide
                                # has_written clear done by the start matmul
                                bass_rust.add_dep_helper(
                                    _inst(r), start_mm, sync=False,
                                    reason="psum bank group order",
                                )
                            for k in range(1, KD):
                                nc.tensor.matmul(
                                    o_ap, wih_t[d][:, k, msl], xt[:, k, :v],
                                    start=False, stop=False,
                                )
                            nc.tensor.matmul(
                                o_ap, bm_t[d][:, msl], mrhs,
                                start=False, stop=False,
                            )
                        starts.append(start_mm)

                    # pass 2: recurrent projection last, so the PE only
                    # stalls on h right before the gates complete
                    for regs in banks:
                        for idx, (off, m) in enumerate(regs):
                            o_ap = ps[:, off : off + v]
                            msl = slice(m * 128, (m + 1) * 128)
                            for kk in range(KH):
                                nc.tensor.matmul(
                                    o_ap,
                                    whh_t[d][:, kk, msl],
                                    h_t[d][:, kk * NS : kk * NS + v],
                                    start=False,
                                    stop=(idx == len(regs) - 1 and kk == KH - 1),
                                )

                    # ACT/DVE on strided multi-range views that skip the
                    # dead gaps between half-ranges
                    def v2(ap, q):
                        return ap.rearrange("p (q n) -> p q n", q=q)[:, :, :v]

                    tg = apool.tile([128, 512], F32, tag=f"tg_{d}", name=f"tg_{d}")
                    nc.scalar.activation(v2(tg[:], 2), v2(tg_sl, 2), AF.Tanh)
                    si = apool.tile([128, 2 * 512], F32, tag=f"si_{d}", name=f"si_{d}")
                    nc.scalar.activation(v2(si[:], 4), v2(si_sl, 4), AF.Sigmoid)
                    so = apool.tile([128, 512], F32, tag=f"so_{d}", name=f"so_{d}")
                    nc.scalar.activation(v2(so[:], 2), v2(so_sl, 2), AF.Sigmoid)

                    t1 = apool.tile([128, 512], F32, tag=f"t1_{d}", name=f"t1_{d}")
                    cc = c_t[d]
                    nc.vector.tensor_tensor(
                        v2(t1[:], 2), v2(si[:, 0:512], 2), v2(tg[:], 2), OP.mult
                    )
                    nc.vector.tensor_tensor(
                        v2(cc[:], 2), v2(si[:, 512:1024], 2), v2(cc[:], 2), OP.mult
                    )
                    nc.vector.tensor_tensor(
                        v2(cc[:], 2), v2(cc[:], 2), v2(t1[:], 2), OP.add
                    )
                    tcn = apool.tile([128, 512], F32, tag=f"tc_{d}", name=f"tc_{d}")
                    nc.scalar.activation(v2(tcn[:], 2), v2(cc[:], 2), AF.Tanh)
                    if last:
                        hf = opool.tile([128, 512], F32, tag=f"hout_{d}", name=f"hout_{d}")
                        nc.vector.tensor_tensor(hf[:], so[:], tcn[:], OP.mult)
                        nc.sync.dma_start(out_d[d][:], hf[:])
                    else:
                        nc.vector.tensor_tensor(
                            v2(h_t[d][:], 2), v2(so[:], 2), v2(tcn[:], 2), OP.mult
                        )

    nc.compile()
    return nc


def _get_nc(t_steps, V):
    key = (t_steps, tuple(V))
    if key not in _NC_CACHE:
        _NC_CACHE[key] = _build(t_steps, V)
    return _NC_CACHE[key]


def _prep_weights(W_ih, W_hh, b):
    """lhsT layouts for one direction."""
    import ml_dtypes

    wdt = ml_dtypes.bfloat16
    wih = np.ascontiguousarray(
        W_ih.T.reshape(128, KD, FH).astype(wdt)
    )  # (p, k) <-> dd = KD*p + k
    whh = np.ascontiguousarray(
        W_hh.T.reshape(KH, 128, FH).transpose(1, 0, 2).astype(wdt)
    )  # (p, kk) <-> hrow = 128*kk + p
    coef = np.zeros(FH, np.float32)
    coef[: 2 * H] = FORCE       # i, f gates
    coef[3 * H :] = FORCE       # o gate
    bm = np.zeros((128, FH), np.float32)
    bm[0] = b.astype(np.float32)
    bm[1] = coef
    bm = np.ascontiguousarray(bm.astype(wdt))
    return wih, whh, bm


def _prep_core(seqs_c, lens_c, t_steps):
    """Per-core device arrays. seqs_c [NS, T, D], lens_c [NS] (sorted desc)."""
    import ml_dtypes

    bf16 = ml_dtypes.bfloat16
    ns = seqs_c.shape[0]
    shift = t_steps - lens_c  # pad steps per sequence
    src_t = np.arange(t_steps)[None, :] - shift[:, None]      # [NS, t]
    valid = src_t >= 0
    gat = seqs_c[np.arange(ns)[:, None], np.clip(src_t, 0, T - 1)]
    xf = np.where(valid[..., None], gat, np.float32(0.0))     # right-aligned
    xb = seqs_c[:, t_steps - 1 :: -1, :]                      # time-reversed

    def to_dev(x_ntd):
        # [NS, t, D] -> [t, 128, KD, NS] with dd = KD*p + k
        xt = x_ntd.transpose(1, 2, 0).astype(bf16)            # [t, D, NS]
        return np.ascontiguousarray(xt.reshape(t_steps, 128, KD, ns))

    maskinv = (np.arange(t_steps)[:, None] < shift[None, :]).astype(np.float32)
    maskrhs = np.zeros((128, t_steps * ns), np.float32)
    maskrhs[0] = 1.0
    maskrhs[1] = maskinv.reshape(t_steps * ns)
    maskrhs = np.ascontiguousarray(maskrhs.astype(bf16))
    return {"xf": to_dev(xf), "xb": to_dev(xb), "maskrhs": maskrhs}


def _unfold(hT):
    """[128, KH*NS] device tile -> [NS, H] h matrix."""
    h_rows = np.concatenate([hT[:, i * NS : (i + 1) * NS] for i in range(KH)], axis=0)
    return h_rows.T  # [NS, H]


def _run(inputs, trace=False, t_cap=None, **spmd_kwargs):
    import ml_dtypes

    all_embs = np.asarray(inputs["all_embs"], dtype=np.float32)
    lengths = np.asarray(inputs["lengths"]).astype(np.int64)
    starts = np.asarray(inputs["starts"]).astype(np.int64)

    if np.array_equal(starts, np.arange(N, dtype=np.int64) * T):
        seqs = all_embs.reshape(N, T, D)
    else:
        seqs = all_embs[starts[:, None] + np.arange(T)[None, :]]

    # global sort by length desc, deal round-robin to cores
    order = np.argsort(-lengths, kind="stable")
    t_steps = int(lengths.max())
    if t_cap is not None:
        t_steps = min(t_steps, t_cap)
    core_idx = [order[c::NCORES] for c in range(NCORES)]  # [NCORES][NS]

    # baked active widths: V_s = max over cores of #{len >= t_steps - s}
    Ls = np.stack([np.minimum(lengths[ci], t_steps) for ci in core_idx])  # [NC, NS]
    thr = t_steps - np.arange(t_steps)  # [t]
    V = (Ls[:, None, :] >= thr[None, :, None]).sum(-1).max(0)  # [t]
    V = np.maximum(V, 1)

    w = {}
    for d, (wi, wh, bb) in {
        "f": (inputs["W_ih_f"], inputs["W_hh_f"], inputs["b_f"]),
        "b": (inputs["W_ih_b"], inputs["W_hh_b"], inputs["b_b"]),
    }.items():
        w[d] = _prep_weights(
            np.asarray(wi, np.float32), np.asarray(wh, np.float32),
            np.asarray(bb, np.float32),
        )

    in_maps = []
    for ci in range(NCORES):
        idx = core_idx[ci]
        m = _prep_core(seqs[idx], np.minimum(lengths[idx], t_steps), t_steps)
        in_maps.append(
            {
                "xf": m["xf"], "xb": m["xb"], "maskrhs": m["maskrhs"],
                "wihf": w["f"][0], "whhf": w["f"][1], "bmf": w["f"][2],
                "wihb": w["b"][0], "whhb": w["b"][1], "bmb": w["b"][2],
            }
        )

    nc = _get_nc(t_steps, V)
    res = None
    for attempt in range(3):
        try:
            res = run_bass_kernel_spmd(
                nc, in_maps, core_ids=list(range(NCORES)), trace=trace,
                **spmd_kwargs
            )
            break
        except Exception:
            # rare transient NRT_EXEC_UNIT_UNRECOVERABLE right after a
            # fresh NEFF load; a plain re-execute has always recovered
            if attempt == 2:
                raise
            import time as _time

            _time.sleep(2.0)

    out = np.empty((N, 2 * H), np.float32)
    for ci in range(NCORES):
        out[core_idx[ci], :H] = _unfold(res.results[ci]["hTf"])
        out[core_idx[ci], H:] = _unfold(res.results[ci]["hTb"])
    return out, res


def kernel(**inputs) -> np.ndarray:
    out, _ = _run(inputs)
    return out
